# revision 34
# baseline (speedup 1.0000x reference)
"""Classical self-attention (head-summed scores) on 8 trn2 NeuronCores.

Math (per batch b):
    S = x Wq (x Wk)^T / 8      (full-E contraction: heads+dims summed)
    P = softmax(S, axis=-1)
    out = P x Wv W_out + b_out

Because the scores contract over the FULL embedding (heads are summed),
the weights fold on the host (weight-only preprocessing, done once):
    GT = Wq Wk^T   ->  S^T = x_keys (GT^T x_q^T)     [query-side first]
    H  = Wv W_out  ->  out = (P x) H + b_out         [x-weighted attn]
so the per-core device work is only 12.9 GF instead of 21.5 GF:
    T1 = GT-transform of the 1024 queries     (2.15 GF)
    S^T = x_keys . T1                         (4.3 GF)
    PXT = x^T P~^T, accumulated transposed    (4.3 GF)
    y   = PXT^T H * recip + b                 (2.15 GF)

Sharding: 8 cores = (4 batches) x (2 query-halves). Each core gets its
batch's x (natural + pre-transposed, bf16-cast on host) rotated so its
1024 query rows come first; keys are the full 2048 rows (key order is
irrelevant). No collectives.

Everything is SBUF-resident; matmul moving operands are bf16 or f32r
with free dim 512, so all matmuls run at 1 cycle/row. T1/PXT stay f32
for precision. One shared PSUM ring of four [128,1024] f32 tiles rotates
through all phases; row sums are ones-matmuls dropped into the previous
s-tile's consumed PSUM columns. Softmax normalization is deferred to the
output stage: one fused DVE op (yps*recip + bias) per half-tile.
"""

import sys

sys.path.insert(0, "/opt/trn_rl_repo")

import numpy as np

import concourse.bass as bass
import concourse.mybir as mybir
import concourse.tile as tile
from concourse import bacc

B, N, E = 4, 2048, 1024
NQ = N // 2          # query rows per core
P = 128              # partitions
FT = E // P          # 8 feature tiles
MT = N // P          # 16 key tiles
QT = NQ // P         # 8 query tiles
SB = 2               # key superblocks of 1024
SBW = N // SB        # superblock width (1024)
HW = SBW // 2        # 512: max psum-bank-safe fp32 matmul width
BF16 = mybir.dt.bfloat16
F32 = mybir.dt.float32
F32R = mybir.dt.float32r
ExpF = mybir.ActivationFunctionType.Exp


def build_program():
    nc = bacc.Bacc("TRN2", target_bir_lowering=False, debug=False)
    xT_d = nc.dram_tensor("xT", [E, N], BF16, kind="ExternalInput").ap()
    xn_d = nc.dram_tensor("xn", [N, E], BF16, kind="ExternalInput").ap()
    gT_d = nc.dram_tensor("gT", [E, E], BF16, kind="ExternalInput").ap()
    h_d = nc.dram_tensor("h", [E, E], BF16, kind="ExternalInput").ap()
    bout = nc.dram_tensor("bout", [E], F32, kind="ExternalInput").ap()
    y = nc.dram_tensor("y", [NQ, E], BF16, kind="ExternalOutput").ap()

    with tile.TileContext(nc) as tc:
        _body(nc, tc, xT_d, xn_d, gT_d, h_d, bout, y)
    nc.compile()
    return nc


class PsumHalves:
    """Eight [128, 512] f32 PSUM tiles (one bank each), shared by every
    phase via one rotation -- no pool is ever released mid-program, so no
    matmul ever write-waits on a pool boundary."""

    def __init__(self, tc):
        self.pool = tc.alloc_tile_pool(name="ps", bufs=1, space="PSUM")
        self.i = 0

    def half(self):
        t = self.pool.tile([P, HW], F32, name=f"ps{self.i & 7}",
                           tag=f"ps{self.i & 7}")
        self.i += 1
        return t


def _body(nc, tc, xT_d, xn_d, gT_d, h_d, bout, y):
    smp = tc.alloc_tile_pool(name="small", bufs=1, side="right")
    ones = smp.tile([P, 1], BF16, name="ones", tag="ones")
    sums_acc = smp.tile([P, QT], F32, name="sums_acc", tag="sums_acc")
    recip = smp.tile([P, QT], F32, name="recip", tag="recip")
    nc.vector.memset(ones, 1.0)

    # Long-lived SBUF tensors (everything fits; nothing is released until
    # the end except the GT staging pool).
    hp = tc.alloc_tile_pool(name="Hp", bufs=1)
    H = [hp.tile([P, E], BF16, name=f"H{c}", tag=f"H{c}") for c in range(FT)]
    bo_b = hp.tile([P, E], F32, name="bo_b", tag="bo_b")
    xtp = tc.alloc_tile_pool(name="xtp", bufs=1)
    xt = [[xtp.tile([P, SBW], BF16, name=f"xt{s}_{f}", tag=f"xt{s}_{f}")
           for f in range(FT)] for s in range(SB)]
    t1p = tc.alloc_tile_pool(name="t1p", bufs=1)
    T1 = [t1p.tile([P, NQ], BF16, name=f"T1_{c}", tag=f"T1_{c}")
          for c in range(FT)]
    xnp = tc.alloc_tile_pool(name="xnp", bufs=1)
    xn = [xnp.tile([P, E], BF16, name=f"xn{m}", tag=f"xn{m}")
          for m in range(MT)]
    pp = tc.alloc_tile_pool(name="pp", bufs=1)
    p_tiles = [pp.tile([P, NQ], BF16, name=f"p{m}", tag=f"p{m}")
               for m in range(MT)]
    pxp = tc.alloc_tile_pool(name="pxp", bufs=1)
    PXT = [pxp.tile([P, NQ], BF16, name=f"PXT{c}", tag=f"PXT{c}")
           for c in range(FT)]
    gtp = tc.alloc_tile_pool(name="gtp", bufs=1)
    GT = [gtp.tile([P, E], BF16, name=f"GT{d}", tag=f"GT{d}")
          for d in range(FT)]

    # --- DMA issue order = need order --------------------------------
    # Phase A consumes one (GT[d], xt0[d] half) pair per 1.7us; Pool
    # (SWDGE) carries GT at ~1.07us/issue and SP (HWDGE) the xt halves
    # at ~0.6us/issue, so the two issue paths feed the wire in parallel.
    for f in range(FT):
        if f == 0:
            # first tile in halves on the fast HWDGE path: the first
            # d-loop touches c=0..3 (h0 cols) well before c=4..7
            for g in range(2):
                nc.sync.dma_start(out=GT[0][:, g * HW:(g + 1) * HW],
                                  in_=gT_d[0:P, g * HW:(g + 1) * HW])
        else:
            nc.gpsimd.dma_start(out=GT[f], in_=gT_d[f * P:(f + 1) * P, :])
        nc.sync.dma_start(out=xt[0][f][:, 0:HW],
                          in_=xT_d[f * P:(f + 1) * P, 0:HW])
    for f in range(FT):
        nc.sync.dma_start(out=xt[0][f][:, HW:SBW],
                          in_=xT_d[f * P:(f + 1) * P, HW:SBW])
    for f in range(FT):
        nc.sync.dma_start(out=xt[1][f],
                          in_=xT_d[f * P:(f + 1) * P, SBW:2 * SBW])
    for c in range(FT):
        nc.gpsimd.dma_start(out=H[c], in_=h_d[c * P:(c + 1) * P, :])
    for m in range(MT):
        nc.gpsimd.dma_start(out=xn[m], in_=xn_d[m * P:(m + 1) * P, :])
    bout_bcast = bass.AP(tensor=bout.tensor, offset=0, ap=[[0, P], [1, E]])
    nc.sync.dma_start(out=bo_b, in_=bout_bcast)

    ps = PsumHalves(tc)

    # --- Phase A: T1 = GT^T xq^T  (d-major over half-width psums) ----
    # 8 half-psums hold all c-tiles of one query half, so each arriving
    # (GT[d], xt half) pair unlocks a full 1.7us of PE work.
    t1ps = [ps.half() for _ in range(FT)]
    # PE clock warmup: dummy matmuls (ones read through a 0-stride
    # free AP, result discarded) span the first-DMA latency window so
    # the p-state ramp burns off before real work arrives.
    ones_rhs = bass.AP(tensor=ones.tensor, offset=ones.offset,
                       ap=[[ones.ap[0][0], P], [0, HW]])
    ones_lhs = bass.AP(tensor=ones.tensor, offset=ones.offset,
                       ap=[[ones.ap[0][0], P], [0, P]])
    for _ in range(6):
        nc.tensor.matmul(t1ps[0], ones_lhs, ones_rhs,
                         start=True, stop=True)

    def copy_out(c, h):
        # alternate DVE/ACT so trailing copies drain 2x as fast
        dst = T1[c][:, h * HW:(h + 1) * HW]
        if c & 1:
            nc.scalar.activation(
                dst, t1ps[c], mybir.ActivationFunctionType.Copy)
        else:
            nc.vector.tensor_copy(dst, t1ps[c])

    # h0 pass: d-major across all c so PE demand (1.7us per d) tracks
    # the DMA supply rate of (GT[d], xt half) pairs.
    for d in range(FT):
        for c in range(FT):
            nc.tensor.matmul(t1ps[c], GT[d][:, c * P:(c + 1) * P],
                             xt[0][d][:, 0:HW],
                             start=(d == 0), stop=(d == FT - 1))
    for c in range(FT):
        copy_out(c, 0)
    # h1 pass: tiles all resident now; c-groups so most copies drain
    # during compute and only the last two trail into phase B.
    for cg in ((0, 1, 2), (3, 4, 5), (6, 7)):
        for d in range(FT):
            for c in cg:
                nc.tensor.matmul(t1ps[c], GT[d][:, c * P:(c + 1) * P],
                                 xt[0][d][:, HW:SBW],
                                 start=(d == 0), stop=(d == FT - 1))
        for c in cg:
            copy_out(c, 1)
    gtp.release()

    # --- Phase B: S^T per key tile; exp on ACT; row sums --------------
    s_tiles = []
    for m in range(MT):
        sb, mloc = divmod(m, SBW // P)
        sh = [ps.half(), ps.half()]
        s_tiles.append(sh[0])
        for f in range(FT):
            for h in range(2):
                nc.tensor.matmul(sh[h],
                                 xt[sb][f][:, mloc * P:(mloc + 1) * P],
                                 T1[f][:, h * HW:(h + 1) * HW],
                                 start=(f == 0), stop=(f == FT - 1))
        for h in range(2):
            nc.scalar.activation(p_tiles[m][:, h * HW:(h + 1) * HW],
                                 sh[h], ExpF, scale=0.125)
        # Row-sum the PREVIOUS tile's exp (ACT ran during this tile's S
        # matmuls); park the sums in that s-tile's consumed PSUM space.
        if m > 0:
            _row_sums(nc, p_tiles[m - 1], s_tiles[m - 1], ones, sums_acc,
                      m - 1)
    _row_sums(nc, p_tiles[MT - 1], s_tiles[MT - 1], ones, sums_acc, MT - 1)
    nc.vector.reciprocal(recip, sums_acc)

    # --- Phase D: PXT[c] = sum_m xn[m,c]^T P~^T[m]  (transposed) ------
    for c in range(FT):
        pxh = [ps.half(), ps.half()]
        for m in range(MT):
            for h in range(2):
                nc.tensor.matmul(pxh[h],
                                 xn[m][:, c * P:(c + 1) * P],
                                 p_tiles[m][:, h * HW:(h + 1) * HW],
                                 start=(m == 0), stop=(m == MT - 1))
        nc.vector.tensor_copy(PXT[c][:, 0:HW], pxh[0])
        nc.scalar.activation(PXT[c][:, HW:NQ], pxh[1],
                             mybir.ActivationFunctionType.Copy)

    # --- Phase F: y = PXT^T H * recip + b ----------------------------
    # One ring tile per half so h1's matmuls never WAR-wait on h0's DVE.
    with tc.tile_pool(name="ysb", bufs=4) as ysp:
        for qt in range(QT):
            for h in range(2):
                yps = ps.half()
                for c in range(FT):
                    nc.tensor.matmul(yps,
                                     PXT[c][:, qt * P:(qt + 1) * P],
                                     H[c][:, h * HW:(h + 1) * HW],
                                     start=(c == 0), stop=(c == FT - 1))
                ysb = ysp.tile([P, HW], BF16, name="ysb", tag="ysb")
                nc.vector.scalar_tensor_tensor(
                    out=ysb, in0=yps,
                    scalar=recip[:, qt:qt + 1],
                    in1=bo_b[:, h * HW:(h + 1) * HW],
                    op0=mybir.AluOpType.mult, op1=mybir.AluOpType.add)
                nc.sync.dma_start(
                    out=y[qt * P:(qt + 1) * P, h * HW:(h + 1) * HW], in_=ysb)

    pxp.release()
    pp.release()
    xnp.release()
    t1p.release()
    xtp.release()
    hp.release()
    smp.release()
    ps.pool.release()


def _row_sums(nc, p, s_prev, ones, sums_acc, m):
    for q in range(QT):
        nc.tensor.matmul(s_prev[:, q:q + 1], p[:, q * P:(q + 1) * P], ones,
                         start=True, stop=True)
    if m == 0:
        nc.vector.tensor_copy(sums_acc, s_prev[:, 0:QT])
    else:
        nc.vector.tensor_tensor(out=sums_acc, in0=sums_acc,
                                in1=s_prev[:, 0:QT], op=mybir.AluOpType.add)


_NC_CACHE = None


def _get_program():
    global _NC_CACHE
    if _NC_CACHE is None:
        _NC_CACHE = build_program()
    return _NC_CACHE


def kernel(x, W_qkv, W_out, b_out):
    from concourse.bass_utils import run_bass_kernel_spmd
    import ml_dtypes

    bf16 = ml_dtypes.bfloat16
    x = np.asarray(x, dtype=np.float32)
    W_qkv = np.asarray(W_qkv, dtype=np.float32)
    W_out = np.asarray(W_out, dtype=np.float32)
    bout32 = np.ascontiguousarray(np.asarray(b_out, dtype=np.float32))

    Wq, Wk, Wv = W_qkv[:, :E], W_qkv[:, E:2 * E], W_qkv[:, 2 * E:]
    gT16 = (Wq @ Wk.T).astype(bf16)    # GT[d,c] = G[c,d], G = Wk Wq^T
    h16 = (Wv @ W_out).astype(bf16)

    nc = _get_program()
    in_maps = []
    xb16 = [x[b].astype(bf16) for b in range(B)]
    for core in range(8):
        b, half = divmod(core, 2)
        xb = xb16[b]
        s = half * NQ
        xrot = np.ascontiguousarray(np.concatenate([xb[s:], xb[:s]], axis=0))
        xrotT = np.ascontiguousarray(xrot.T)
        in_maps.append({"xT": xrotT, "xn": xrot, "gT": gT16, "h": h16,
                        "bout": bout32})
    res = run_bass_kernel_spmd(nc, in_maps, list(range(8)))
    out = np.empty((B, N, E), dtype=np.float32)
    for core in range(8):
        b, half = divmod(core, 2)
        out[b, half * NQ:(half + 1) * NQ] = res.results[core]["y"].astype(np.float32)
    return out


# revision 35
# speedup vs baseline: 1.0040x; 1.0040x over previous
"""Classical self-attention (head-summed scores) on 8 trn2 NeuronCores.

Math (per batch b):
    S = x Wq (x Wk)^T / 8      (full-E contraction: heads+dims summed)
    P = softmax(S, axis=-1)
    out = P x Wv W_out + b_out

Because the scores contract over the FULL embedding (heads are summed),
the weights fold on the host (weight-only preprocessing, done once):
    GT = Wq Wk^T   ->  S^T = x_keys (GT^T x_q^T)     [query-side first]
    H  = Wv W_out  ->  out = (P x) H + b_out         [x-weighted attn]
so the per-core device work is only 12.9 GF instead of 21.5 GF:
    T1 = GT-transform of the 1024 queries     (2.15 GF)
    S^T = x_keys . T1                         (4.3 GF)
    PXT = x^T P~^T, accumulated transposed    (4.3 GF)
    y   = PXT^T H * recip + b                 (2.15 GF)

Sharding: 8 cores = (4 batches) x (2 query-halves). Each core gets its
batch's x (natural + pre-transposed, bf16-cast on host) rotated so its
1024 query rows come first; keys are the full 2048 rows (key order is
irrelevant). No collectives.

Everything is SBUF-resident; matmul moving operands are bf16 or f32r
with free dim 512, so all matmuls run at 1 cycle/row. T1/PXT stay f32
for precision. One shared PSUM ring of four [128,1024] f32 tiles rotates
through all phases; row sums are ones-matmuls dropped into the previous
s-tile's consumed PSUM columns. Softmax normalization is deferred to the
output stage: one fused DVE op (yps*recip + bias) per half-tile.
"""

import sys

sys.path.insert(0, "/opt/trn_rl_repo")

import numpy as np

import concourse.bass as bass
import concourse.mybir as mybir
import concourse.tile as tile
from concourse import bacc

B, N, E = 4, 2048, 1024
NQ = N // 2          # query rows per core
P = 128              # partitions
FT = E // P          # 8 feature tiles
MT = N // P          # 16 key tiles
QT = NQ // P         # 8 query tiles
SB = 2               # key superblocks of 1024
SBW = N // SB        # superblock width (1024)
HW = SBW // 2        # 512: max psum-bank-safe fp32 matmul width
BF16 = mybir.dt.bfloat16
F32 = mybir.dt.float32
F32R = mybir.dt.float32r
ExpF = mybir.ActivationFunctionType.Exp


def build_program():
    nc = bacc.Bacc("TRN2", target_bir_lowering=False, debug=False)
    xT_d = nc.dram_tensor("xT", [E, N], BF16, kind="ExternalInput").ap()
    xn_d = nc.dram_tensor("xn", [N, E], BF16, kind="ExternalInput").ap()
    gT_d = nc.dram_tensor("gT", [E, E], BF16, kind="ExternalInput").ap()
    h_d = nc.dram_tensor("h", [E, E], BF16, kind="ExternalInput").ap()
    bout = nc.dram_tensor("bout", [E], F32, kind="ExternalInput").ap()
    y = nc.dram_tensor("y", [NQ, E], BF16, kind="ExternalOutput").ap()

    with tile.TileContext(nc) as tc:
        _body(nc, tc, xT_d, xn_d, gT_d, h_d, bout, y)
    nc.compile()
    return nc


class PsumHalves:
    """Eight [128, 512] f32 PSUM tiles (one bank each), shared by every
    phase via one rotation -- no pool is ever released mid-program, so no
    matmul ever write-waits on a pool boundary."""

    def __init__(self, tc):
        self.pool = tc.alloc_tile_pool(name="ps", bufs=1, space="PSUM")
        self.i = 0

    def half(self):
        t = self.pool.tile([P, HW], F32, name=f"ps{self.i & 7}",
                           tag=f"ps{self.i & 7}")
        self.i += 1
        return t


def _body(nc, tc, xT_d, xn_d, gT_d, h_d, bout, y):
    smp = tc.alloc_tile_pool(name="small", bufs=1, side="right")
    ones = smp.tile([P, 1], BF16, name="ones", tag="ones")
    sums_acc = smp.tile([P, QT], F32, name="sums_acc", tag="sums_acc")
    recip = smp.tile([P, QT], F32, name="recip", tag="recip")
    nc.vector.memset(ones, 1.0)

    # Long-lived SBUF tensors (everything fits; nothing is released until
    # the end except the GT staging pool).
    hp = tc.alloc_tile_pool(name="Hp", bufs=1)
    H = [hp.tile([P, E], BF16, name=f"H{c}", tag=f"H{c}") for c in range(FT)]
    bo_b = hp.tile([P, E], F32, name="bo_b", tag="bo_b")
    xtp = tc.alloc_tile_pool(name="xtp", bufs=1)
    xt = [[xtp.tile([P, SBW], BF16, name=f"xt{s}_{f}", tag=f"xt{s}_{f}")
           for f in range(FT)] for s in range(SB)]
    t1p = tc.alloc_tile_pool(name="t1p", bufs=1)
    T1 = [t1p.tile([P, NQ], BF16, name=f"T1_{c}", tag=f"T1_{c}")
          for c in range(FT)]
    xnp = tc.alloc_tile_pool(name="xnp", bufs=1)
    xn = [xnp.tile([P, E], BF16, name=f"xn{m}", tag=f"xn{m}")
          for m in range(MT)]
    pp = tc.alloc_tile_pool(name="pp", bufs=1)
    p_tiles = [pp.tile([P, NQ], BF16, name=f"p{m}", tag=f"p{m}")
               for m in range(MT)]
    pxp = tc.alloc_tile_pool(name="pxp", bufs=1)
    PXT = [pxp.tile([P, NQ], BF16, name=f"PXT{c}", tag=f"PXT{c}")
           for c in range(FT)]
    gtp = tc.alloc_tile_pool(name="gtp", bufs=1)
    GT = [gtp.tile([P, E], BF16, name=f"GT{d}", tag=f"GT{d}")
          for d in range(FT)]

    # --- DMA issue order = need order --------------------------------
    # Phase A consumes one (GT[d], xt0[d] half) pair per 1.7us; Pool
    # (SWDGE) carries GT at ~1.07us/issue and SP (HWDGE) the xt halves
    # at ~0.6us/issue, so the two issue paths feed the wire in parallel.
    for f in range(FT):
        if f == 0:
            # first tile in halves: the first d-loop touches c=0..3 (h0
            # cols) almost a microsecond before c=4..7
            for g in range(2):
                nc.gpsimd.dma_start(out=GT[0][:, g * HW:(g + 1) * HW],
                                    in_=gT_d[0:P, g * HW:(g + 1) * HW])
        else:
            nc.gpsimd.dma_start(out=GT[f], in_=gT_d[f * P:(f + 1) * P, :])
        nc.sync.dma_start(out=xt[0][f][:, 0:HW],
                          in_=xT_d[f * P:(f + 1) * P, 0:HW])
    for f in range(FT):
        nc.sync.dma_start(out=xt[0][f][:, HW:SBW],
                          in_=xT_d[f * P:(f + 1) * P, HW:SBW])
    for f in range(FT):
        nc.sync.dma_start(out=xt[1][f],
                          in_=xT_d[f * P:(f + 1) * P, SBW:2 * SBW])
    for c in range(FT):
        nc.gpsimd.dma_start(out=H[c], in_=h_d[c * P:(c + 1) * P, :])
    for m in range(MT):
        nc.gpsimd.dma_start(out=xn[m], in_=xn_d[m * P:(m + 1) * P, :])
    bout_bcast = bass.AP(tensor=bout.tensor, offset=0, ap=[[0, P], [1, E]])
    nc.sync.dma_start(out=bo_b, in_=bout_bcast)

    ps = PsumHalves(tc)

    # --- Phase A: T1 = GT^T xq^T  (d-major over half-width psums) ----
    # 8 half-psums hold all c-tiles of one query half, so each arriving
    # (GT[d], xt half) pair unlocks a full 1.7us of PE work.
    t1ps = [ps.half() for _ in range(FT)]
    # PE clock warmup: dummy matmuls (ones read through a 0-stride
    # free AP, result discarded) span the first-DMA latency window so
    # the p-state ramp burns off before real work arrives.
    ones_rhs = bass.AP(tensor=ones.tensor, offset=ones.offset,
                       ap=[[ones.ap[0][0], P], [0, HW]])
    ones_lhs = bass.AP(tensor=ones.tensor, offset=ones.offset,
                       ap=[[ones.ap[0][0], P], [0, P]])
    for _ in range(6):
        nc.tensor.matmul(t1ps[0], ones_lhs, ones_rhs,
                         start=True, stop=True)

    def copy_out(c, h):
        # alternate DVE/ACT so trailing copies drain 2x as fast
        dst = T1[c][:, h * HW:(h + 1) * HW]
        if c & 1:
            nc.scalar.activation(
                dst, t1ps[c], mybir.ActivationFunctionType.Copy)
        else:
            nc.vector.tensor_copy(dst, t1ps[c])

    # h0 pass: d-major across all c so PE demand (1.7us per d) tracks
    # the DMA supply rate of (GT[d], xt half) pairs.
    for d in range(FT):
        for c in range(FT):
            nc.tensor.matmul(t1ps[c], GT[d][:, c * P:(c + 1) * P],
                             xt[0][d][:, 0:HW],
                             start=(d == 0), stop=(d == FT - 1))
    for c in range(FT):
        copy_out(c, 0)
    # h1 pass: tiles all resident now; c-groups so most copies drain
    # during compute and only the last two trail into phase B.
    for cg in ((0, 1, 2), (3, 4, 5), (6, 7)):
        for d in range(FT):
            for c in cg:
                nc.tensor.matmul(t1ps[c], GT[d][:, c * P:(c + 1) * P],
                                 xt[0][d][:, HW:SBW],
                                 start=(d == 0), stop=(d == FT - 1))
        for c in cg:
            copy_out(c, 1)
    gtp.release()

    # --- Phase B: S^T per key tile; exp on ACT; row sums --------------
    s_tiles = []
    for m in range(MT):
        sb, mloc = divmod(m, SBW // P)
        sh = [ps.half(), ps.half()]
        s_tiles.append(sh[0])
        for f in range(FT):
            for h in range(2):
                nc.tensor.matmul(sh[h],
                                 xt[sb][f][:, mloc * P:(mloc + 1) * P],
                                 T1[f][:, h * HW:(h + 1) * HW],
                                 start=(f == 0), stop=(f == FT - 1))
        for h in range(2):
            nc.scalar.activation(p_tiles[m][:, h * HW:(h + 1) * HW],
                                 sh[h], ExpF, scale=0.125)
        # Row-sum the PREVIOUS tile's exp (ACT ran during this tile's S
        # matmuls); park the sums in that s-tile's consumed PSUM space.
        if m > 0:
            _row_sums(nc, p_tiles[m - 1], s_tiles[m - 1], ones, sums_acc,
                      m - 1)
    _row_sums(nc, p_tiles[MT - 1], s_tiles[MT - 1], ones, sums_acc, MT - 1)
    nc.vector.reciprocal(recip, sums_acc)

    # --- Phase D: PXT[c] = sum_m xn[m,c]^T P~^T[m]  (transposed) ------
    for c in range(FT):
        pxh = [ps.half(), ps.half()]
        for m in range(MT):
            for h in range(2):
                nc.tensor.matmul(pxh[h],
                                 xn[m][:, c * P:(c + 1) * P],
                                 p_tiles[m][:, h * HW:(h + 1) * HW],
                                 start=(m == 0), stop=(m == MT - 1))
        nc.vector.tensor_copy(PXT[c][:, 0:HW], pxh[0])
        nc.scalar.activation(PXT[c][:, HW:NQ], pxh[1],
                             mybir.ActivationFunctionType.Copy)

    # --- Phase F: y = PXT^T H * recip + b ----------------------------
    # One ring tile per half so h1's matmuls never WAR-wait on h0's DVE.
    with tc.tile_pool(name="ysb", bufs=4) as ysp:
        for qt in range(QT):
            for h in range(2):
                yps = ps.half()
                for c in range(FT):
                    nc.tensor.matmul(yps,
                                     PXT[c][:, qt * P:(qt + 1) * P],
                                     H[c][:, h * HW:(h + 1) * HW],
                                     start=(c == 0), stop=(c == FT - 1))
                ysb = ysp.tile([P, HW], BF16, name="ysb", tag="ysb")
                nc.vector.scalar_tensor_tensor(
                    out=ysb, in0=yps,
                    scalar=recip[:, qt:qt + 1],
                    in1=bo_b[:, h * HW:(h + 1) * HW],
                    op0=mybir.AluOpType.mult, op1=mybir.AluOpType.add)
                nc.sync.dma_start(
                    out=y[qt * P:(qt + 1) * P, h * HW:(h + 1) * HW], in_=ysb)

    pxp.release()
    pp.release()
    xnp.release()
    t1p.release()
    xtp.release()
    hp.release()
    smp.release()
    ps.pool.release()


def _row_sums(nc, p, s_prev, ones, sums_acc, m):
    for q in range(QT):
        nc.tensor.matmul(s_prev[:, q:q + 1], p[:, q * P:(q + 1) * P], ones,
                         start=True, stop=True)
    if m == 0:
        nc.vector.tensor_copy(sums_acc, s_prev[:, 0:QT])
    else:
        nc.vector.tensor_tensor(out=sums_acc, in0=sums_acc,
                                in1=s_prev[:, 0:QT], op=mybir.AluOpType.add)


_NC_CACHE = None


def _get_program():
    global _NC_CACHE
    if _NC_CACHE is None:
        _NC_CACHE = build_program()
    return _NC_CACHE


def kernel(x, W_qkv, W_out, b_out):
    from concourse.bass_utils import run_bass_kernel_spmd
    import ml_dtypes

    bf16 = ml_dtypes.bfloat16
    x = np.asarray(x, dtype=np.float32)
    W_qkv = np.asarray(W_qkv, dtype=np.float32)
    W_out = np.asarray(W_out, dtype=np.float32)
    bout32 = np.ascontiguousarray(np.asarray(b_out, dtype=np.float32))

    Wq, Wk, Wv = W_qkv[:, :E], W_qkv[:, E:2 * E], W_qkv[:, 2 * E:]
    gT16 = (Wq @ Wk.T).astype(bf16)    # GT[d,c] = G[c,d], G = Wk Wq^T
    h16 = (Wv @ W_out).astype(bf16)

    nc = _get_program()
    in_maps = []
    xb16 = [x[b].astype(bf16) for b in range(B)]
    for core in range(8):
        b, half = divmod(core, 2)
        xb = xb16[b]
        s = half * NQ
        xrot = np.ascontiguousarray(np.concatenate([xb[s:], xb[:s]], axis=0))
        xrotT = np.ascontiguousarray(xrot.T)
        in_maps.append({"xT": xrotT, "xn": xrot, "gT": gT16, "h": h16,
                        "bout": bout32})
    res = run_bass_kernel_spmd(nc, in_maps, list(range(8)))
    out = np.empty((B, N, E), dtype=np.float32)
    for core in range(8):
        b, half = divmod(core, 2)
        out[b, half * NQ:(half + 1) * NQ] = res.results[core]["y"].astype(np.float32)
    return out
